# revision 5
# baseline (speedup 1.0000x reference)
"""Depthwise causal Conv1d (k=4) + SiLU on 8 Trainium2 NeuronCores.

Problem: x [4, 4096, 2048] f32, w [2048, 4] f32,
out[b, t, d] = silu(sum_j w[d, j] * x[b, t - 3 + j, d])   (zero-padded left).

Sharding: 8 cores = 4 batches x 2 channel-halves. Depthwise conv is
independent per channel, so channel sharding needs no halo exchange.

Layout: each core receives its shard host-transposed to [channels, time]
(channels on SBUF partitions). The per-channel weight w[d, j] is a
per-partition scalar and the causal time shifts are free-dim AP offsets
into one loaded tile.

The kernel is HBM-bandwidth-bound (~16.8 MB of fp16 I/O per core against
~358 GB/s per-NeuronCore HBM), so the schedule is built around keeping
the 16 SDMA engines saturated end to end:
 - All 8 input-block loads are queued back to back on the sync HWDGE
   ring with nothing else on it, so the load stream runs at full HBM
   rate and finishes as early as possible (~24 us).
 - Stores (SWDGE on GpSimd) are gated behind a tiny GpSimd op that
   reads the second-to-last x tile: the store ring stays empty while
   loads are in flight (strict load priority instead of packet-level
   fair sharing), then the SBUF output backlog drains at full rate.
 - Compute is sized well under the DMA window and split across engines:
   even blocks run on DVE (4 tensor_scalar products into a packed
   [128,2,L] pair layout, one [2,L] pair-add, final add in halves),
   odd blocks run on the TensorEngine as diag(w_j) matmuls accumulating
   the 4 taps in PSUM. ACT does only SiLU. The diag(w_j) matrices are
   built on-chip from a 32 KB host-sent identity (one tensor_scalar_mul
   each) instead of the 1 MB DRAM load a host-built diag would need.

Precision: x and the output are host-cast fp16 (halves HBM traffic both
ways); products and adds stay fp16 (PE accumulates fp32 in PSUM); SiLU
computes fp32-internally on ACT. End-to-end relative error ~5e-4.
"""

import sys
import types

import numpy as np

import concourse.bass as bass
import concourse.bacc as bacc
import concourse.mybir as mybir
from concourse.tile import TileContext
from concourse.bass_utils import run_bass_kernel_spmd


def _ensure_ntff_hook():
    """bass_utils imports antenv.axon_hooks when BASS_TRACE is set; that
    module is absent on this image. Install a shim so tracing works when
    possible and degrades gracefully (instead of crashing) when not."""
    try:
        import antenv.axon_hooks  # noqa: F401

        return
    except ImportError:
        pass
    try:
        import antenv

        hook = None
        try:
            if "/root/.axon_site" not in sys.path:
                sys.path.insert(0, "/root/.axon_site")
            from trn_agent_boot.trn_boot import _ntff_profile_via_ctypes

            hook = _ntff_profile_via_ctypes("/opt/axon/libaxon_pjrt.so")
        except Exception:
            hook = None
        mod = types.ModuleType("antenv.axon_hooks")
        mod._hook = hook
        mod.get_axon_ntff_profile_hook = lambda: mod._hook
        mod.set_axon_ntff_profile_hook = lambda h: setattr(mod, "_hook", h)
        sys.modules["antenv.axon_hooks"] = mod
        antenv.axon_hooks = mod
    except Exception:
        pass


_ensure_ntff_hook()

B, L, D = 4, 4096, 2048
K = 4
PAD = K - 1
N_CORES = 8
DH = D // 2            # channels per core
NBLK = DH // 128       # 128-partition channel blocks per core
ROWW = 4128            # DRAM row stride (fp16 elems): 64B-aligned rows

MID_DT = mybir.dt.float16
PE_BLKS = (1, 3, 5, 7)  # blocks computed on the TensorEngine

_cache = {}


def _build_bass():
    nc = bacc.Bacc()
    xt = nc.dram_tensor("xt", [DH, ROWW], MID_DT, kind="ExternalInput")
    wt = nc.dram_tensor("wt", [128, NBLK * K], mybir.dt.float32, kind="ExternalInput")
    ident = nc.dram_tensor("ident", [128, 128], MID_DT, kind="ExternalInput")
    ot = nc.dram_tensor("ot", [DH, L], MID_DT, kind="ExternalOutput")
    f32 = mybir.dt.float32
    HALF = L // 2

    with TileContext(nc) as tc:
        with tc.tile_pool(name="pool", bufs=2) as pool, \
             tc.tile_pool(name="psum", bufs=2, space="PSUM") as psum_pool:
            # w and the identity lead the sync ring: on a second ring they
            # starve behind the x-load packets and stall all compute.
            w = pool.tile([128, NBLK * K], f32, tag="w", bufs=1)
            nc.sync.dma_start(out=w[:], in_=wt[:, :])
            idt = pool.tile([128, 128], MID_DT, tag="idt", bufs=1)
            nc.sync.dma_start(out=idt[:], in_=ident[:, :])
            # Warmup: a tiny Silu forces the silu activation-table set to
            # load during the initial DMA wait; it is the only table load
            # in the whole kernel.
            warm = pool.tile([128, 2], MID_DT, tag="warm", bufs=1)
            nc.vector.memset(warm[:], 0.0)
            nc.scalar.activation(warm[:], warm[:], mybir.ActivationFunctionType.Silu)

            # All loads up front, back to back on the sync ring.
            xts = []
            for blk in range(NBLK):
                x = pool.tile([128, L + PAD + 1], MID_DT, tag="x", bufs=NBLK)
                nc.sync.dma_start(
                    out=x[:, 0 : L + PAD],
                    in_=xt[blk * 128 : (blk + 1) * 128, 0 : L + PAD],
                )
                xts.append(x)

            # Build diag(w_j) for the PE blocks on-chip: one per-partition
            # scaling of the identity each.
            wdt = pool.tile([128, len(PE_BLKS) * K * 128], MID_DT, tag="wd", bufs=1)
            wd_col = {}
            c = 0
            for blk in PE_BLKS:
                for j in range(K):
                    nc.vector.tensor_scalar_mul(
                        wdt[:, c : c + 128], idt[:], w[:, blk * K + j : blk * K + j + 1]
                    )
                    wd_col[(blk, j)] = c
                    c += 128

            outs = []
            for blk in range(NBLK):
                x = xts[blk]
                wj = lambda j: w[:, blk * K + j : blk * K + j + 1]
                o = pool.tile([128, L], MID_DT, tag="o", bufs=NBLK)
                if blk in PE_BLKS:
                    # TensorEngine path: per 1024-col PSUM quarter,
                    # accumulate the 4 diag(w_j) matmuls (shift = free-dim
                    # offset on the moving operand), SiLU out of PSUM.
                    PQ = 1024
                    for q in range(L // PQ):
                        h0 = q * PQ
                        ps = psum_pool.tile([128, PQ], f32, tag="ps", bufs=4)
                        for j in range(K):
                            lw = wdt[:, wd_col[(blk, j)] : wd_col[(blk, j)] + 128]
                            for cc in range(PQ // 512):
                                nc.tensor.matmul(
                                    ps[:, cc * 512 : (cc + 1) * 512],
                                    lw,
                                    x[:, h0 + cc * 512 + j : h0 + cc * 512 + j + 512],
                                    start=(j == 0),
                                    stop=(j == K - 1),
                                )
                        nc.scalar.activation(
                            o[:, h0 : h0 + PQ], ps[:],
                            mybir.ActivationFunctionType.Silu,
                        )
                else:
                    # DVE path: tap-0 product, then three fused
                    # multiply-accumulates (affine_then_add custom DVE op:
                    # out = in0*scale + bias + in1), shift-rebased so
                    # acc[:, t] accumulates w_j * x[:, t + j]. Halves so
                    # ACT can start SiLU while the second half accumulates.
                    acc = pool.tile([128, L], MID_DT, tag="acc", bufs=2)
                    nc.vector.tensor_scalar_mul(acc[:], x[:, 0:L], wj(0))
                    for h in range(2):
                        s = h * HALF
                        for j in range(1, K):
                            nc.vector.affine_then_add(
                                acc[:, s : s + HALF],
                                x[:, s + j : s + j + HALF],
                                acc[:, s : s + HALF],
                                wj(j),
                                0.0,
                            )
                        nc.scalar.activation(
                            o[:, s : s + HALF], acc[:, s : s + HALF],
                            mybir.ActivationFunctionType.Silu,
                        )
                outs.append(o)

            # Store gate: this GpSimd op reads the second-to-last x tile, so
            # (GpSimd being in-order) no store descriptor reaches the SWDGE
            # ring until the load stream is nearly done. Gating on x6 rather
            # than x7 hides the ~3 us GpSimd op latency behind x7's drain.
            dep = pool.tile([128, 1], MID_DT, tag="dep", bufs=1)
            nc.gpsimd.tensor_scalar_add(dep[:], xts[6][:, L + PAD - 1 : L + PAD], 0.0)
            for blk in range(NBLK):
                nc.gpsimd.dma_start(
                    out=ot[blk * 128 : (blk + 1) * 128, 0:L], in_=outs[blk][:]
                )
    nc.compile()
    return nc


def _shard_inputs(x, w):
    ident = np.ascontiguousarray(np.eye(128, dtype=np.float16))
    in_maps = []
    for core in range(N_CORES):
        b, half = divmod(core, 2)
        d0 = half * DH
        xt = np.zeros((DH, ROWW), dtype=np.float16)
        xt[:, PAD : PAD + L] = x[b, :, d0 : d0 + DH].T.astype(np.float16)
        # w rows for this shard, rearranged so partition p holds the K
        # weights of channel blk*128 + p at free cols [blk*K, blk*K + K)
        w_sh = w[d0 : d0 + DH].reshape(NBLK, 128, K)
        wt = (
            w_sh.transpose(1, 0, 2).reshape(128, NBLK * K).astype(np.float32)
        )
        in_maps.append(
            {
                "xt": np.ascontiguousarray(xt),
                "wt": np.ascontiguousarray(wt),
                "ident": ident,
            }
        )
    return in_maps


def kernel(x, w):
    x = np.asarray(x, dtype=np.float32)
    w = np.asarray(w, dtype=np.float32)
    assert x.shape == (B, L, D) and w.shape == (D, K)

    if "nc" not in _cache:
        _cache["nc"] = _build_bass()
    nc = _cache["nc"]

    in_maps = _shard_inputs(x, w)
    res = None
    for attempt in range(3):
        try:
            res = run_bass_kernel_spmd(nc, in_maps, core_ids=list(range(N_CORES)))
            break
        except Exception:
            if attempt == 2:
                raise
    _cache["last_results"] = res

    out = np.empty((B, L, D), dtype=np.float32)
    for core in range(N_CORES):
        b, half = divmod(core, 2)
        d0 = half * DH
        out[b, :, d0 : d0 + DH] = res.results[core]["ot"].T.astype(np.float32)
    return out
